# revision 42
# baseline (speedup 1.0000x reference)
"""Trainium2 Bass kernel for nn_AttentionBlock (GroupNorm + MHA + proj + residual).

Sharding: data-parallel over batch (16 batches -> 2 per core x 8 cores).
Weights replicated. Each core computes its 2 batches fully; host gathers.

Per-batch dataflow on a core (c=512, t=1024, H=8 heads, dh=64, 32 groups):
  x [512,1024] -> GroupNorm (stats via DVE; cross-partition group aggregation
      and scale/bias broadcast via tiny indicator matmuls; rsqrt via DVE
      bit-trick + Newton; no DRAM round trips, no ScalarE) -> xn (bf16)
  qk = Wqk_reordered @ xn   (8 o-tiles; pair-ordered so head-pairs share tiles)
  vT = xn^T @ Wv^T          (v produced transposed: [s, c_v], ones col per head)
  per head-pair per (s-tile, t-half): logitsT [s, (2 heads x 512t)] via two
      K=64 matmuls at PE row-tiles (0,0)/(64,0) -> run concurrently;
      one exp per tile on ScalarE (PSUM->SBUF bf16)
  attnRaw[c'=65, t] = vAugT^T @ wT  (65th row = softmax denominator)
  denominator: DVE reciprocal -> bf16 -> broadcast to 128 partitions via an
      indicator matmul (no DRAM round trip); attn = attnRaw * recip
  out = w_proj @ attn + b_proj + x
"""

import os
import sys

os.environ.setdefault("MYCRO_LOCAL_CACHE", "1")
for _p in ("/root/.axon_site", "/root/.axon_site/_ro/trn_rl_repo",
           "/root/.axon_site/_ro/pypackages", "/opt/trn_rl_repo"):
    if os.path.isdir(_p) and _p not in sys.path:
        sys.path.append(_p)

import numpy as np

from concourse import bass, bacc, tile, mybir
from concourse._compat import get_trn_type
from concourse.bass_utils import run_bass_kernel_spmd

F32 = mybir.dt.float32
I32 = mybir.dt.int32
BF16 = mybir.dt.bfloat16

N_CORES = 8
B, C, HH, WW = 16, 512, 32, 32
T = HH * WW            # 1024
NHEADS = 8
DH = C // NHEADS       # 64
NGROUPS = 32
GSIZE = C // NGROUPS   # 16 channels per group
EPS = 1e-5
BPC = B // N_CORES     # batches per core = 2
P = 128
NPAIR = NHEADS // 2    # 4 head pairs
CT = C // P            # 4 channel tiles
OT = (2 * C) // P      # 8 qk output tiles
ST = T // P            # 8 s-tiles
TH = T // 512          # 2 t-halves

LAST_RESULTS = None


def _bc_ap(ap, nparts):
    """Broadcast an AP along a new leading partition dim of size nparts."""
    return bass.AP(tensor=ap.tensor, offset=ap.offset,
                   ap=[[0, nparts]] + [list(d) for d in ap.ap])


def build_nc():
    nc = bacc.Bacc(get_trn_type() or "TRN2", target_bir_lowering=False,
                   debug=False)

    xs_d = nc.dram_tensor("xs", [BPC, C, T], BF16, kind="ExternalInput")
    wqkT_d = nc.dram_tensor("wqkT", [C, 2 * C], BF16, kind="ExternalInput")
    bqkT_d = nc.dram_tensor("bqkT", [P, OT], F32, kind="ExternalInput")
    wvT_d = nc.dram_tensor("wvT", [C, C], BF16, kind="ExternalInput")
    bvA_d = nc.dram_tensor("bvA", [NHEADS * 65], F32, kind="ExternalInput")
    wpT_d = nc.dram_tensor("wpT", [C, C], BF16, kind="ExternalInput")
    bpT_d = nc.dram_tensor("bpT", [P, CT], F32, kind="ExternalInput")
    gnT_d = nc.dram_tensor("gnT", [P, CT * 2], F32, kind="ExternalInput")
    i8_d = nc.dram_tensor("i8", [P, 8], F32, kind="ExternalInput")
    ib_d = nc.dram_tensor("ib", [8, P], F32, kind="ExternalInput")
    ir_d = nc.dram_tensor("ir", [65, P], BF16, kind="ExternalInput")
    out_d = nc.dram_tensor("out", [BPC, C, T], F32, kind="ExternalOutput")

    from contextlib import ExitStack
    with ExitStack() as ctx:
        tc = ctx.enter_context(tile.TileContext(nc))
        cpool = ctx.enter_context(tc.tile_pool(name="const", bufs=1))
        xpool = ctx.enter_context(tc.tile_pool(name="xp", bufs=8))
        xnpool = ctx.enter_context(tc.tile_pool(name="xnp", bufs=8))
        qkpool = ctx.enter_context(tc.tile_pool(name="qkp", bufs=16))
        vtpool = ctx.enter_context(tc.tile_pool(name="vtp", bufs=16))
        wtpool = ctx.enter_context(tc.tile_pool(name="wtp", bufs=24))
        attnpool = ctx.enter_context(tc.tile_pool(name="attnp", bufs=8))
        outpool = ctx.enter_context(tc.tile_pool(name="outp", bufs=4))
        smallpool = ctx.enter_context(tc.tile_pool(name="smallp", bufs=2))
        recpool = ctx.enter_context(tc.tile_pool(name="recp", bufs=2))
        ps_mm = ctx.enter_context(tc.tile_pool(name="ps_mm", bufs=2, space="PSUM"))
        ps_lg = ctx.enter_context(tc.tile_pool(name="ps_lg", bufs=2, space="PSUM"))
        ps_at = ctx.enter_context(tc.tile_pool(name="ps_at", bufs=2, space="PSUM"))

        EXP = mybir.ActivationFunctionType.Exp
        ALU = mybir.AluOpType

        # ---- one-time constant loads (scalar-engine DMA queue, ordered so
        #      the bytes needed first transfer first; x loads go on the sync
        #      queue with x0 ahead of x1) ----
        i8_sb = cpool.tile([P, 8], F32, tag="i8")
        nc.scalar.dma_start(i8_sb[:], i8_d[:])
        ib_sb = cpool.tile([8, P], F32, tag="ib")
        nc.scalar.dma_start(ib_sb[:], ib_d[:])
        gnT_sb = cpool.tile([P, CT, 2], F32, tag="gnT")
        nc.scalar.dma_start(gnT_sb[:], gnT_d[:])
        bqk_sb = cpool.tile([P, OT], F32, tag="bqk")
        nc.scalar.dma_start(bqk_sb[:], bqkT_d[:])

        # x tiles next: spread over all three DMA-capable queues right
        # behind the tiny constants so x0 gets the full HBM bandwidth
        def load_x(b):
            xs = []
            for j in range(CT):
                xt = xpool.tile([P, T], BF16, tag="x")
                eng = (nc.sync, nc.gpsimd, nc.scalar, nc.scalar)[j]
                eng.dma_start(xt[:], xs_d[b, P * j:P * (j + 1), :])
                xs.append(xt)
            return xs

        x0 = load_x(0)
        x1 = load_x(1)

        wqkT_sb = []
        for k in range(CT):
            w = cpool.tile([P, 2 * C], BF16, tag=f"wqkT{k}")
            nc.scalar.dma_start(w[:], wqkT_d[P * k:P * (k + 1), :])
            wqkT_sb.append(w)
        wvT_sb = []
        for k in range(CT):
            w = cpool.tile([P, C], BF16, tag=f"wvT{k}")
            nc.scalar.dma_start(w[:], wvT_d[P * k:P * (k + 1), :])
            wvT_sb.append(w)
        # late-needed constants at the back of the scalar queue
        bv_bc = cpool.tile([P, NHEADS * 65], F32, tag="bv")
        nc.scalar.dma_start(bv_bc[:], _bc_ap(bvA_d.ap(), P))
        ir_sb = cpool.tile([65, P], BF16, tag="ir")
        nc.scalar.dma_start(ir_sb[:], ir_d[:])
        bp_sb = cpool.tile([P, CT], F32, tag="bp")
        nc.scalar.dma_start(bp_sb[:], bpT_d[:])
        wpT_sb = []
        for k in range(CT):
            w = cpool.tile([P, C], BF16, tag=f"wpT{k}")
            nc.scalar.dma_start(w[:], wpT_d[P * k:P * (k + 1), :])
            wpT_sb.append(w)

        # ---------- per-batch emission helpers ----------
        def emit_gn(x_sb):
            """GroupNorm: DVE stats, tiny-matmul partition aggregation and
            broadcast, DVE bit-trick rsqrt. No DRAM trips, no ScalarE.
            Stats matmuls are emitted per tile (pipelining with the x DMA),
            but the rsqrt chain runs once for all 32 groups so the in-order
            tensor queue never serializes against the DVE chain."""
            pst = ps_mm.tile([P, 512], F32, tag="psmm")
            for j in range(CT):
                bst = smallpool.tile([P, 2, 6], F32, tag="bst")
                for sg in range(2):
                    nc.vector.bn_stats(out=bst[:, sg, :],
                                       in_=x_sb[j][:, 512 * sg:512 * (sg + 1)])
                mv3 = smallpool.tile([P, 3], F32, tag="mv3")
                nc.vector.bn_aggr(out=mv3[:, 0:2], in_=bst[:])
                nc.vector.tensor_mul(mv3[:, 2:3], mv3[:, 0:1], mv3[:, 0:1])
                # group-of-16 partition sums of (mean, var, mean^2)
                nc.tensor.matmul(pst[0:8, 3 * j:3 * (j + 1)], i8_sb[:],
                                 mv3[:], start=True, stop=True)

            pg = pst[0:8, 0:3 * CT].rearrange("p (j v) -> p j v", v=3)
            gm = smallpool.tile([8, CT, 3], F32, tag="gm")
            nc.vector.tensor_scalar_mul(gm[:], pg, 1.0 / GSIZE)
            u = smallpool.tile([8, CT], F32, tag="u")  # E[x^2] per group
            nc.vector.tensor_add(u[:], gm[:, :, 1], gm[:, :, 2])
            musq8 = smallpool.tile([8, CT], F32, tag="musq8")
            nc.vector.tensor_mul(musq8[:], gm[:, :, 0], gm[:, :, 0])
            veps = smallpool.tile([8, CT], F32, tag="veps")  # var + eps
            nc.vector.scalar_tensor_tensor(
                out=veps[:], in0=u[:], scalar=EPS,
                op0=ALU.add, in1=musq8[:], op1=ALU.subtract)

            # rsqrt(veps): magic-constant guess + one Newton iteration
            y = smallpool.tile([8, CT], F32, tag="rsq")
            ib32 = smallpool.tile([8, CT], I32, tag="ib32")
            nc.vector.tensor_scalar(
                out=ib32[:], in0=veps[:].bitcast(I32), scalar1=1,
                scalar2=None, op0=ALU.logical_shift_right)
            nc.vector.tensor_scalar(
                out=y[:].bitcast(I32), in0=ib32[:], scalar1=-1,
                scalar2=0x5f3759df, op0=ALU.mult, op1=ALU.add)
            nt = smallpool.tile([8, CT], F32, tag="nt")
            for _ in range(2):
                nc.vector.tensor_mul(nt[:], y[:], y[:])
                nc.vector.scalar_tensor_tensor(
                    out=nt[:], in0=nt[:], scalar=-0.5,
                    op0=ALU.mult, in1=veps[:], op1=ALU.mult)
                nc.vector.tensor_scalar_add(nt[:], nt[:], 1.5)
                nc.vector.tensor_mul(y[:], y[:], nt[:])
                break  # one Newton step: 4.6e-6 rel err, under bf16 noise

            # broadcast (rsqrt, mu) to channel partitions via indicator
            # matmuls (all emitted together), then a/b and the xn applies
            for j in range(CT):
                rm = smallpool.tile([8, 2], F32, tag="rm")
                nc.vector.tensor_copy(rm[:, 0:1], y[:, j:j + 1])
                nc.vector.tensor_copy(rm[:, 1:2], gm[:, j, 0:1])
                nc.tensor.matmul(pst[:, 16 + 2 * j:16 + 2 * (j + 1)],
                                 ib_sb[:], rm[:], start=True, stop=True)
            xn_sb = []
            for j in range(CT):
                pb = pst[:, 16 + 2 * j:16 + 2 * (j + 1)]
                ab = smallpool.tile([P, 2], F32, tag="ab")
                nc.vector.tensor_mul(ab[:, 0:1], gnT_sb[:, j, 0:1], pb[:, 0:1])
                t2 = smallpool.tile([P, 1], F32, tag="t2")
                nc.vector.tensor_mul(t2[:], pb[:, 1:2], ab[:, 0:1])
                nc.vector.tensor_sub(ab[:, 1:2], gnT_sb[:, j, 1:2], t2[:])
                xn = xnpool.tile([P, T], BF16, tag="xn")
                nc.vector.tensor_scalar(
                    out=xn[:], in0=x_sb[j][:], scalar1=ab[:, 0:1],
                    scalar2=ab[:, 1:2], op0=ALU.mult, op1=ALU.add)
                xn_sb.append(xn)
            return xn_sb

        def qkv_groups(xn_sb, qk_sb, vt_sb, pair0_first):
            """Closures, each emitting one PSUM group (~4 MMs + evac)."""
            def qk_group(j, th):
                def emit():
                    if th == 0:
                        qk = qkpool.tile([P, T], BF16, tag="qk")
                        qk_sb.append(qk)
                        assert qk_sb[j] is qk
                    qk = qk_sb[j]
                    ps = ps_mm.tile([P, 512], F32, tag="psmm")
                    for k in range(CT):
                        nc.tensor.matmul(
                            ps[:], wqkT_sb[k][:, P * j:P * (j + 1)],
                            xn_sb[k][:, 512 * th:512 * (th + 1)],
                            start=(k == 0), stop=(k == CT - 1))
                    nc.vector.tensor_scalar_add(
                        qk[:, 512 * th:512 * (th + 1)], ps[:],
                        bqk_sb[:, j:j + 1])
                return emit

            def vt_group(st):
                def emit():
                    vt = vtpool.tile([P, NHEADS * 65], BF16, tag="vt")
                    vt_sb.append(vt)
                    vt3 = vt[:].rearrange("p (h c) -> p h c", h=NHEADS)
                    ps = ps_mm.tile([P, 512], F32, tag="psmm")
                    for k in range(CT):
                        nc.tensor.matmul(
                            ps[:], xn_sb[k][:, P * st:P * (st + 1)],
                            wvT_sb[k][:],
                            start=(k == 0), stop=(k == CT - 1))
                    bv3 = bv_bc[:].rearrange("p (h c) -> p h c", h=NHEADS)
                    nc.vector.tensor_add(
                        vt3[:, :, 0:DH],
                        ps[:].rearrange("p (h c) -> p h c", h=NHEADS),
                        bv3[:, :, 0:DH])
                    nc.vector.tensor_copy(vt3[:, :, DH:DH + 1],
                                          bv3[:, :, DH:DH + 1])
                return emit

            groups = []
            if pair0_first:
                # pair-0 q/k first (unblocks first logits: the th0 halves of
                # both q and k before anything else), then v (needed by the
                # first attention slot), then the rest of q/k
                for th in range(TH):
                    for j in (0, 1):
                        groups.append(qk_group(j, th))
                for st in range(ST):
                    groups.append(vt_group(st))
                for j in range(2, OT):
                    for th in range(TH):
                        groups.append(qk_group(j, th))
            else:
                for j in range(OT):
                    for th in range(TH):
                        groups.append(qk_group(j, th))
                for st in range(ST):
                    groups.append(vt_group(st))
            return groups

        def proj_groups(b, attn_sb, x_sb, th_major=False):
            groups = []
            ots = {}

            def pgroup(j, th):
                def emit():
                    if th == 0:
                        ots[j] = outpool.tile([P, T], F32, tag="out",
                                              name="out_t")
                    ot = ots[j]
                    ps = ps_mm.tile([P, 512], F32, tag="psmm")
                    for k in range(CT):
                        nc.tensor.matmul(
                            ps[:], wpT_sb[k][:, P * j:P * (j + 1)],
                            attn_sb[k][:, 512 * th:512 * (th + 1)],
                            start=(k == 0), stop=(k == CT - 1))
                    nc.vector.scalar_tensor_tensor(
                        out=ot[:, 512 * th:512 * (th + 1)], in0=ps[:],
                        scalar=bp_sb[:, j:j + 1],
                        in1=x_sb[j][:, 512 * th:512 * (th + 1)],
                        op0=ALU.add, op1=ALU.add)
                    if b == 0:
                        if th == 1:
                            nc.gpsimd.dma_start(
                                out_d[b, P * j:P * (j + 1), :], ot[:])
                    else:
                        # tail batch: store each half as soon as it is
                        # ready, round-robin over all three DMA-capable
                        # queues so the tail drains fast
                        eng = (nc.sync, nc.scalar, nc.gpsimd)[(2 * j + th) % 3]
                        eng.dma_start(
                            out_d[b, P * j:P * (j + 1),
                                  512 * th:512 * (th + 1)],
                            ot[:, 512 * th:512 * (th + 1)])
                return emit

            if th_major:
                order = [(j, th) for th in range(TH) for j in range(CT)]
            else:
                order = [(j, th) for j in range(CT) for th in range(TH)]
            for j, th in order:
                groups.append(pgroup(j, th))
            return groups

        def emit_lg_exp(qk_sb, p_i, st, th, wts):
            """One logits tile [s=128, (2 heads x 512 t)] + one exp.
            The two heads' K=64 matmuls sit at PE row-tiles (0,0)/(64,0)
            and execute concurrently."""
            qt = qk_sb[2 * p_i]
            kt = qk_sb[2 * p_i + 1]
            lg = ps_lg.tile([P, T], F32, tag="pslg")
            for hh in range(2):
                lo = 64 * hh
                nc.tensor.matmul(
                    lg[:, 512 * hh:512 * (hh + 1)],
                    kt[lo:lo + DH, P * st:P * (st + 1)],
                    qt[lo:lo + DH, 512 * th:512 * (th + 1)],
                    start=True, stop=True)
            wt = wtpool.tile([P, T], BF16, tag="wt")
            nc.scalar.activation(wt[:], lg[:], EXP, bias=0.0)
            wts[(st, th)] = wt

        # ---------- software-pipelined schedule ----------
        pair_ids = [(b, p) for b in range(BPC) for p in range(NPAIR)]

        xn0 = emit_gn(x0)
        qk0, vt0 = [], []
        qkv0 = qkv_groups(xn0, qk0, vt0, pair0_first=True)
        for g in qkv0[:4]:      # qk j0, j1 — unblocks pair-0 logits
            g()
        # prefetch logits/exp of the first pair (ahead of v/qk-rest in the
        # in-order tensor queue so ScalarE starts as early as possible)
        wts_cur = {}
        for th in range(TH):
            for st in range(ST):
                emit_lg_exp(qk0, 0, st, th, wts_cur)
        for g in qkv0[4:12]:    # vt st0-7 (needed by the first attn slot)
            g()
        # batch-1 GroupNorm emitted only now so its DVE work lands behind
        # the startup-critical GN0 -> qk -> logits chain in the static order
        xn1 = emit_gn(x1)
        qk1, vt1 = [], []
        # qk j2..j7 of batch 0 + all of batch-1 qkv run as slot fillers
        fillers = qkv0[12:] + qkv_groups(xn1, qk1, vt1, pair0_first=False)

        xs = {0: x0, 1: x1}
        qks = {0: qk0, 1: qk1}
        vts = {0: vt0, 1: vt1}
        attns = {0: [], 1: []}
        pending = []          # deferred divide work of the previous pair

        for idx, (b, p_i) in enumerate(pair_ids):
            last = idx == len(pair_ids) - 1
            vt_sb = vts[b]
            at = attnpool.tile([P, T], BF16, tag="attn")
            recF = recpool.tile([65, T], F32, tag="recF")
            nc.gpsimd.memset(recF[:], 1.0)
            recR = recpool.tile([65, T], F32, tag="recR")
            recB = recpool.tile([65, T], BF16, tag="recB")

            def div_chain(th):
                sl = slice(512 * th, 512 * (th + 1))
                rb = ps_mm.tile([P, 512], F32, tag="psmm")
                nc.tensor.matmul(rb[:], ir_sb[:], recB[:, sl],
                                 start=True, stop=True)
                nc.vector.tensor_mul(at[:, sl], at[:, sl], rb[:])

            wts_next = {}
            nb, np_i = (pair_ids[idx + 1]
                        if idx + 1 < len(pair_ids) else (None, None))
            # next-pair (st, th) emission order: all of th0 during the first
            # two slots (matches wt-tile death of the current pair's th0)
            SLOT_LG = [[(st, 0) for st in range(4)],
                       [(st, 0) for st in range(4, 8)],
                       [(st, 1) for st in range(4)],
                       [(st, 1) for st in range(4, 8)]]

            slot = 0
            for th in range(TH):
                sl = slice(512 * th, 512 * (th + 1))
                for hh in range(2):
                    h_abs = 2 * p_i + hh
                    # make sure the next pair's q/k tiles exist before any
                    # emit_lg_exp references them (batch-boundary case)
                    while (nb is not None and fillers
                           and len(qks[nb]) < 2 * np_i + 2):
                        fillers.pop(0)()
                    lgq = list(SLOT_LG[slot]) if nb is not None else []
                    pa = ps_at.tile([65, 512], F32, tag="psat")
                    for st in range(ST):
                        # interleave next-pair logits+exp in groups of two —
                        # enough to keep ScalarE fed while halving the
                        # PE row-mode (64x128 <-> 128x128) switch count
                        if st in (0, 4):
                            for _ in range(2):
                                if lgq:
                                    st2, th2 = lgq.pop(0)
                                    emit_lg_exp(qks[nb], np_i, st2, th2,
                                                wts_next)
                        nc.tensor.matmul(
                            pa[:],
                            vt_sb[st][:, 65 * h_abs:65 * (h_abs + 1)],
                            wts_cur[(st, th)][:, 512 * hh:512 * (hh + 1)],
                            start=(st == 0), stop=(st == ST - 1))
                    nc.vector.tensor_copy(at[64 * hh:64 * hh + DH, sl],
                                          pa[0:DH, :])
                    nc.vector.tensor_copy(recF[64 * hh:64 * hh + 1, sl],
                                          pa[DH:DH + 1, :])
                    # deferred divides of the previous pair (all at once —
                    # partial pops can deadlock the in-order tensor queue
                    # against filler groups that read the attn tiles)
                    while pending:
                        pending.pop(0)()
                    npop = 2 if len(fillers) > 8 else 1
                    for _ in range(npop):
                        if fillers:
                            fillers.pop(0)()
                    slot += 1
                    # last pair: divide the t0 half as soon as both heads'
                    # t0 slots are done, then start batch-1 proj on that
                    # half under the cover of the t1 attention slots
                    if last and slot == 2:
                        attns[b].append(at)
                        nc.vector.reciprocal_approx_fast(
                            out=recR[:, 0:512], in_=recF[:, 0:512])
                        nc.vector.tensor_copy(recB[:, 0:512], recR[:, 0:512])
                        div_chain(0)
                        pg1 = proj_groups(1, attns[1], x1, th_major=True)
                    if last and slot == 3:
                        for g in pg1[:CT]:
                            g()

            if not last:
                attns[b].append(at)

            if last:
                nc.vector.reciprocal_approx_fast(
                    out=recR[:, 512:T], in_=recF[:, 512:T])
                nc.vector.tensor_copy(recB[:, 512:T], recR[:, 512:T])
                div_chain(1)
                for g in pg1[CT:]:
                    g()
            else:
                # denominator reciprocal now; broadcast matmul + divide
                # deferred into the next pair's slots (keeps the tensor
                # queue stall-free)
                nc.vector.reciprocal_approx_fast(out=recR[:], in_=recF[:])
                nc.vector.tensor_copy(recB[:], recR[:])

                def make_div(at, recB):
                    def run():
                        for th in range(TH):
                            sl = slice(512 * th, 512 * (th + 1))
                            rb = ps_mm.tile([P, 512], F32, tag="psmm")
                            nc.tensor.matmul(rb[:], ir_sb[:], recB[:, sl],
                                             start=True, stop=True)
                            nc.vector.tensor_mul(at[:, sl], at[:, sl], rb[:])
                    return run
                pending.append(make_div(at, recB))
                wts_cur = wts_next

            # end of batch 0's pairs: flush remaining fillers, then queue
            # batch-0 proj as fillers for batch 1's pairs
            if b == 0 and p_i == NPAIR - 1:
                for g in fillers:
                    g()
                fillers = list(proj_groups(0, attns[0], x0))

        # flush any leftover filler work
        for g in fillers:
            g()

    nc.compile()
    return nc


def prep_inputs(x, gn_scale, gn_bias, w_qkv, b_qkv, w_proj, b_proj):
    """Host-side: reorder/scale weights, build per-core input maps."""
    x2 = np.ascontiguousarray(
        np.asarray(x, dtype=np.float32).reshape(B, C, T))
    w_qkv = np.asarray(w_qkv, dtype=np.float32)
    b_qkv = np.asarray(b_qkv, dtype=np.float32)
    scale = float(DH) ** -0.25

    qk_rows = []
    for p_i in range(NPAIR):
        for hh in range(2):           # q rows of the pair
            h = 2 * p_i + hh
            qk_rows.extend(range(192 * h, 192 * h + DH))
        for hh in range(2):           # k rows of the pair
            h = 2 * p_i + hh
            qk_rows.extend(range(192 * h + DH, 192 * h + 2 * DH))
    qk_rows = np.array(qk_rows)
    bf16 = mybir.dt.np(BF16)
    x2 = x2.astype(bf16)
    wqkT = np.ascontiguousarray((w_qkv[qk_rows] * scale).T).astype(bf16)
    bqkT = np.ascontiguousarray(
        (b_qkv[qk_rows] * scale).reshape(OT, P).T)

    v_rows = np.array([192 * h + 2 * DH + j for h in range(NHEADS)
                       for j in range(DH)])
    wvT = np.ascontiguousarray(w_qkv[v_rows].T).astype(bf16)
    bv = b_qkv[v_rows]
    bvA = np.zeros(NHEADS * 65, np.float32)
    for h in range(NHEADS):
        bvA[65 * h:65 * h + DH] = bv[DH * h:DH * (h + 1)]
        bvA[65 * h + DH] = 1.0

    wpT = np.ascontiguousarray(np.asarray(w_proj, np.float32).T).astype(bf16)
    bpT = np.ascontiguousarray(
        np.asarray(b_proj, np.float32).reshape(CT, P).T)
    gnT = np.ascontiguousarray(np.stack(
        [np.asarray(gn_scale, np.float32).reshape(CT, P).T,
         np.asarray(gn_bias, np.float32).reshape(CT, P).T],
        axis=2).reshape(P, CT * 2))

    i8 = np.zeros((P, 8), np.float32)
    for p in range(P):
        i8[p, p // GSIZE] = 1.0
    ib = np.ascontiguousarray(i8.T)
    ir = np.zeros((65, P), np.float32)
    ir[0, 0:64] = 1.0
    ir[64, 64:128] = 1.0
    ir = ir.astype(bf16)

    common = dict(wqkT=wqkT, bqkT=bqkT, wvT=wvT, bvA=bvA, wpT=wpT,
                  bpT=bpT, gnT=gnT, i8=i8, ib=ib, ir=ir)
    in_maps = [dict(common, xs=np.ascontiguousarray(x2[BPC * i:BPC * (i + 1)]))
               for i in range(N_CORES)]
    return in_maps


_NC = None


def _ensure_ntff_hook():
    """The agent image's antenv lacks axon_hooks; shim it and register the
    ctypes NTFF hook from the boot script so trace=True can measure HW time."""
    try:
        from antenv import axon_hooks  # noqa: F401
        return
    except ImportError:
        pass
    import types
    import antenv
    mod = types.ModuleType("antenv.axon_hooks")
    _state = {"fn": None}
    mod.set_axon_ntff_profile_hook = lambda fn: _state.__setitem__("fn", fn)
    mod.get_axon_ntff_profile_hook = lambda: _state["fn"]
    sys.modules["antenv.axon_hooks"] = mod
    antenv.axon_hooks = mod
    try:
        from trn_agent_boot.trn_boot import _ntff_profile_via_ctypes
        hook = _ntff_profile_via_ctypes("/opt/axon/libaxon_pjrt.so")
        mod.set_axon_ntff_profile_hook(hook)
    except Exception as e:  # degrade: run proceeds untraced
        print("ntff hook setup failed:", e)


def kernel(x, gn_scale, gn_bias, w_qkv, b_qkv, w_proj, b_proj):
    global _NC, LAST_RESULTS
    if _NC is None:
        _NC = build_nc()
    in_maps = prep_inputs(x, gn_scale, gn_bias, w_qkv, b_qkv, w_proj, b_proj)
    trace = bool(os.environ.get("KERNEL_TRACE"))
    if trace:
        _ensure_ntff_hook()
    res = run_bass_kernel_spmd(_NC, in_maps, list(range(N_CORES)), trace=trace)
    LAST_RESULTS = res
    out = np.concatenate([res.results[i]["out"] for i in range(N_CORES)],
                         axis=0)
    return out.reshape(B, C, HH, WW).astype(np.float32)


# revision 47
# speedup vs baseline: 1.0141x; 1.0141x over previous
"""Trainium2 Bass kernel for nn_AttentionBlock (GroupNorm + MHA + proj + residual).

Sharding: data-parallel over batch (16 batches -> 2 per core x 8 cores).
Weights replicated. Each core computes its 2 batches fully; host gathers.

Per-batch dataflow on a core (c=512, t=1024, H=8 heads, dh=64, 32 groups):
  x [512,1024] -> GroupNorm (stats via DVE; cross-partition group aggregation
      and scale/bias broadcast via tiny indicator matmuls; rsqrt via DVE
      bit-trick + Newton; no DRAM round trips, no ScalarE) -> xn (bf16)
  qk = Wqk_reordered @ xn   (8 o-tiles; pair-ordered so head-pairs share tiles)
  vT = xn^T @ Wv^T          (v produced transposed: [s, c_v], ones col per head)
  per head-pair per (s-tile, t-half): logitsT [s, (2 heads x 512t)] via two
      K=64 matmuls at PE row-tiles (0,0)/(64,0) -> run concurrently;
      one exp per tile on ScalarE (PSUM->SBUF bf16)
  attnRaw[c'=65, t] = vAugT^T @ wT  (65th row = softmax denominator)
  denominator: DVE reciprocal -> bf16 -> broadcast to 128 partitions via an
      indicator matmul (no DRAM round trip); attn = attnRaw * recip
  out = w_proj @ attn + b_proj + x
"""

import os
import sys

os.environ.setdefault("MYCRO_LOCAL_CACHE", "1")
for _p in ("/root/.axon_site", "/root/.axon_site/_ro/trn_rl_repo",
           "/root/.axon_site/_ro/pypackages", "/opt/trn_rl_repo"):
    if os.path.isdir(_p) and _p not in sys.path:
        sys.path.append(_p)

import numpy as np

from concourse import bass, bacc, tile, mybir
from concourse._compat import get_trn_type
from concourse.bass_utils import run_bass_kernel_spmd

F32 = mybir.dt.float32
I32 = mybir.dt.int32
BF16 = mybir.dt.bfloat16

N_CORES = 8
B, C, HH, WW = 16, 512, 32, 32
T = HH * WW            # 1024
NHEADS = 8
DH = C // NHEADS       # 64
NGROUPS = 32
GSIZE = C // NGROUPS   # 16 channels per group
EPS = 1e-5
BPC = B // N_CORES     # batches per core = 2
P = 128
NPAIR = NHEADS // 2    # 4 head pairs
CT = C // P            # 4 channel tiles
OT = (2 * C) // P      # 8 qk output tiles
ST = T // P            # 8 s-tiles
TH = T // 512          # 2 t-halves

LAST_RESULTS = None


def _bc_ap(ap, nparts):
    """Broadcast an AP along a new leading partition dim of size nparts."""
    return bass.AP(tensor=ap.tensor, offset=ap.offset,
                   ap=[[0, nparts]] + [list(d) for d in ap.ap])


def build_nc():
    nc = bacc.Bacc(get_trn_type() or "TRN2", target_bir_lowering=False,
                   debug=False)

    xs_d = nc.dram_tensor("xs", [BPC, C, T], BF16, kind="ExternalInput")
    wqkT_d = nc.dram_tensor("wqkT", [C, 2 * C], BF16, kind="ExternalInput")
    bqkT_d = nc.dram_tensor("bqkT", [P, OT], F32, kind="ExternalInput")
    wvT_d = nc.dram_tensor("wvT", [C, C], BF16, kind="ExternalInput")
    bvA_d = nc.dram_tensor("bvA", [NHEADS * 65], F32, kind="ExternalInput")
    wpT_d = nc.dram_tensor("wpT", [C, C], BF16, kind="ExternalInput")
    bpT_d = nc.dram_tensor("bpT", [P, CT], F32, kind="ExternalInput")
    gnT_d = nc.dram_tensor("gnT", [P, CT * 2], F32, kind="ExternalInput")
    i8_d = nc.dram_tensor("i8", [P, 8], F32, kind="ExternalInput")
    ib_d = nc.dram_tensor("ib", [8, P], F32, kind="ExternalInput")
    ir_d = nc.dram_tensor("ir", [65, P], BF16, kind="ExternalInput")
    out_d = nc.dram_tensor("out", [BPC, C, T], F32, kind="ExternalOutput")

    from contextlib import ExitStack
    with ExitStack() as ctx:
        tc = ctx.enter_context(tile.TileContext(nc))
        cpool = ctx.enter_context(tc.tile_pool(name="const", bufs=1))
        xpool = ctx.enter_context(tc.tile_pool(name="xp", bufs=8))
        xnpool = ctx.enter_context(tc.tile_pool(name="xnp", bufs=8))
        qkpool = ctx.enter_context(tc.tile_pool(name="qkp", bufs=16))
        vtpool = ctx.enter_context(tc.tile_pool(name="vtp", bufs=16))
        wtpool = ctx.enter_context(tc.tile_pool(name="wtp", bufs=24))
        attnpool = ctx.enter_context(tc.tile_pool(name="attnp", bufs=8))
        outpool = ctx.enter_context(tc.tile_pool(name="outp", bufs=4))
        smallpool = ctx.enter_context(tc.tile_pool(name="smallp", bufs=2))
        recpool = ctx.enter_context(tc.tile_pool(name="recp", bufs=2))
        ps_mm = ctx.enter_context(tc.tile_pool(name="ps_mm", bufs=2, space="PSUM"))
        ps_lg = ctx.enter_context(tc.tile_pool(name="ps_lg", bufs=2, space="PSUM"))
        ps_at = ctx.enter_context(tc.tile_pool(name="ps_at", bufs=2, space="PSUM"))

        EXP = mybir.ActivationFunctionType.Exp
        ALU = mybir.AluOpType

        # ---- one-time constant loads (scalar-engine DMA queue, ordered so
        #      the bytes needed first transfer first; x loads go on the sync
        #      queue with x0 ahead of x1) ----
        i8_sb = cpool.tile([P, 8], F32, tag="i8")
        nc.scalar.dma_start(i8_sb[:], i8_d[:])
        ib_sb = cpool.tile([8, P], F32, tag="ib")
        nc.scalar.dma_start(ib_sb[:], ib_d[:])
        gnT_sb = cpool.tile([P, CT, 2], F32, tag="gnT")
        nc.scalar.dma_start(gnT_sb[:], gnT_d[:])
        bqk_sb = cpool.tile([P, OT], F32, tag="bqk")
        nc.scalar.dma_start(bqk_sb[:], bqkT_d[:])

        # x tiles next: spread over all three DMA-capable queues right
        # behind the tiny constants so x0 gets the full HBM bandwidth
        def load_x(b):
            xs = []
            for j in range(CT):
                xt = xpool.tile([P, T], BF16, tag="x")
                eng = (nc.sync, nc.gpsimd, nc.scalar, nc.scalar)[j]
                eng.dma_start(xt[:], xs_d[b, P * j:P * (j + 1), :])
                xs.append(xt)
            return xs

        x0 = load_x(0)
        x1 = load_x(1)

        wqkT_sb = []
        for k in range(CT):
            w = cpool.tile([P, 2 * C], BF16, tag=f"wqkT{k}")
            nc.scalar.dma_start(w[:], wqkT_d[P * k:P * (k + 1), :])
            wqkT_sb.append(w)
        wvT_sb = []
        for k in range(CT):
            w = cpool.tile([P, C], BF16, tag=f"wvT{k}")
            nc.scalar.dma_start(w[:], wvT_d[P * k:P * (k + 1), :])
            wvT_sb.append(w)
        # late-needed constants at the back of the scalar queue
        bv_bc = cpool.tile([P, NHEADS * 65], F32, tag="bv")
        nc.scalar.dma_start(bv_bc[:], _bc_ap(bvA_d.ap(), P))
        ir_sb = cpool.tile([65, P], BF16, tag="ir")
        nc.scalar.dma_start(ir_sb[:], ir_d[:])
        bp_sb = cpool.tile([P, CT], F32, tag="bp")
        nc.scalar.dma_start(bp_sb[:], bpT_d[:])
        wpT_sb = []
        for k in range(CT):
            w = cpool.tile([P, C], BF16, tag=f"wpT{k}")
            nc.scalar.dma_start(w[:], wpT_d[P * k:P * (k + 1), :])
            wpT_sb.append(w)

        # ---------- per-batch emission helpers ----------
        def emit_gn(x_sb):
            """GroupNorm: DVE stats, tiny-matmul partition aggregation and
            broadcast, DVE bit-trick rsqrt. No DRAM trips, no ScalarE.
            Stats matmuls are emitted per tile (pipelining with the x DMA),
            but the rsqrt chain runs once for all 32 groups so the in-order
            tensor queue never serializes against the DVE chain."""
            pst = ps_mm.tile([P, 512], F32, tag="psmm")
            for j in range(CT):
                bst = smallpool.tile([P, 2, 6], F32, tag="bst")
                for sg in range(2):
                    nc.vector.bn_stats(out=bst[:, sg, :],
                                       in_=x_sb[j][:, 512 * sg:512 * (sg + 1)])
                mv3 = smallpool.tile([P, 3], F32, tag="mv3")
                nc.vector.bn_aggr(out=mv3[:, 0:2], in_=bst[:])
                nc.vector.tensor_mul(mv3[:, 2:3], mv3[:, 0:1], mv3[:, 0:1])
                # group-of-16 partition sums of (mean, var, mean^2)
                nc.tensor.matmul(pst[0:8, 3 * j:3 * (j + 1)], i8_sb[:],
                                 mv3[:], start=True, stop=True)

            pg = pst[0:8, 0:3 * CT].rearrange("p (j v) -> p j v", v=3)
            gm = smallpool.tile([8, CT, 3], F32, tag="gm")
            nc.vector.tensor_scalar_mul(gm[:], pg, 1.0 / GSIZE)
            u = smallpool.tile([8, CT], F32, tag="u")  # E[x^2] per group
            nc.vector.tensor_add(u[:], gm[:, :, 1], gm[:, :, 2])
            musq8 = smallpool.tile([8, CT], F32, tag="musq8")
            nc.vector.tensor_mul(musq8[:], gm[:, :, 0], gm[:, :, 0])
            veps = smallpool.tile([8, CT], F32, tag="veps")  # var + eps
            nc.vector.scalar_tensor_tensor(
                out=veps[:], in0=u[:], scalar=EPS,
                op0=ALU.add, in1=musq8[:], op1=ALU.subtract)

            # rsqrt(veps): magic-constant guess + one Newton iteration
            y = smallpool.tile([8, CT], F32, tag="rsq")
            ib32 = smallpool.tile([8, CT], I32, tag="ib32")
            nc.vector.tensor_scalar(
                out=ib32[:], in0=veps[:].bitcast(I32), scalar1=1,
                scalar2=None, op0=ALU.logical_shift_right)
            nc.vector.tensor_scalar(
                out=y[:].bitcast(I32), in0=ib32[:], scalar1=-1,
                scalar2=0x5f3759df, op0=ALU.mult, op1=ALU.add)
            nt = smallpool.tile([8, CT], F32, tag="nt")
            for _ in range(2):
                nc.vector.tensor_mul(nt[:], y[:], y[:])
                nc.vector.scalar_tensor_tensor(
                    out=nt[:], in0=nt[:], scalar=-0.5,
                    op0=ALU.mult, in1=veps[:], op1=ALU.mult)
                nc.vector.tensor_scalar_add(nt[:], nt[:], 1.5)
                nc.vector.tensor_mul(y[:], y[:], nt[:])
                break  # one Newton step: 4.6e-6 rel err, under bf16 noise

            # broadcast (rsqrt, mu) to channel partitions via indicator
            # matmuls (all emitted together), then a/b and the xn applies
            for j in range(CT):
                rm = smallpool.tile([8, 2], F32, tag="rm")
                nc.vector.tensor_copy(rm[:, 0:1], y[:, j:j + 1])
                nc.vector.tensor_copy(rm[:, 1:2], gm[:, j, 0:1])
                nc.tensor.matmul(pst[:, 16 + 2 * j:16 + 2 * (j + 1)],
                                 ib_sb[:], rm[:], start=True, stop=True)
            xn_sb = []
            for j in range(CT):
                pb = pst[:, 16 + 2 * j:16 + 2 * (j + 1)]
                ab = smallpool.tile([P, 2], F32, tag="ab")
                nc.vector.tensor_mul(ab[:, 0:1], gnT_sb[:, j, 0:1], pb[:, 0:1])
                t2 = smallpool.tile([P, 1], F32, tag="t2")
                nc.vector.tensor_mul(t2[:], pb[:, 1:2], ab[:, 0:1])
                nc.vector.tensor_sub(ab[:, 1:2], gnT_sb[:, j, 1:2], t2[:])
                xn = xnpool.tile([P, T], BF16, tag="xn")
                nc.vector.tensor_scalar(
                    out=xn[:], in0=x_sb[j][:], scalar1=ab[:, 0:1],
                    scalar2=ab[:, 1:2], op0=ALU.mult, op1=ALU.add)
                xn_sb.append(xn)
            return xn_sb

        def qkv_groups(xn_sb, qk_sb, vt_sb, pair0_first):
            """Closures, each emitting one PSUM group (~4 MMs + evac)."""
            def qk_group(j, th):
                def emit():
                    if th == 0:
                        qk = qkpool.tile([P, T], BF16, tag="qk")
                        qk_sb.append(qk)
                        assert qk_sb[j] is qk
                    qk = qk_sb[j]
                    ps = ps_mm.tile([P, 512], F32, tag="psmm")
                    for k in range(CT):
                        nc.tensor.matmul(
                            ps[:], wqkT_sb[k][:, P * j:P * (j + 1)],
                            xn_sb[k][:, 512 * th:512 * (th + 1)],
                            start=(k == 0), stop=(k == CT - 1))
                    nc.vector.tensor_scalar_add(
                        qk[:, 512 * th:512 * (th + 1)], ps[:],
                        bqk_sb[:, j:j + 1])
                return emit

            def vt_group(st):
                def emit():
                    vt = vtpool.tile([P, NHEADS * 65], BF16, tag="vt")
                    vt_sb.append(vt)
                    vt3 = vt[:].rearrange("p (h c) -> p h c", h=NHEADS)
                    ps = ps_mm.tile([P, 512], F32, tag="psmm")
                    for k in range(CT):
                        nc.tensor.matmul(
                            ps[:], xn_sb[k][:, P * st:P * (st + 1)],
                            wvT_sb[k][:],
                            start=(k == 0), stop=(k == CT - 1))
                    bv3 = bv_bc[:].rearrange("p (h c) -> p h c", h=NHEADS)
                    nc.vector.tensor_add(
                        vt3[:, :, 0:DH],
                        ps[:].rearrange("p (h c) -> p h c", h=NHEADS),
                        bv3[:, :, 0:DH])
                    nc.vector.tensor_copy(vt3[:, :, DH:DH + 1],
                                          bv3[:, :, DH:DH + 1])
                return emit

            groups = []
            if pair0_first:
                # pair-0 q/k first (unblocks first logits), then v (needed by
                # the first attention slot), then the rest of q/k
                for j in (0, 1):
                    for th in range(TH):
                        groups.append(qk_group(j, th))
                for st in range(ST):
                    groups.append(vt_group(st))
                for j in range(2, OT):
                    for th in range(TH):
                        groups.append(qk_group(j, th))
            else:
                for j in range(OT):
                    for th in range(TH):
                        groups.append(qk_group(j, th))
                for st in range(ST):
                    groups.append(vt_group(st))
            return groups

        def proj_groups(b, attn_sb, x_sb, th_major=False):
            groups = []
            ots = {}

            def pgroup(j, th):
                def emit():
                    if th == 0:
                        ots[j] = outpool.tile([P, T], F32, tag="out",
                                              name="out_t")
                    ot = ots[j]
                    ps = ps_mm.tile([P, 512], F32, tag="psmm")
                    for k in range(CT):
                        nc.tensor.matmul(
                            ps[:], wpT_sb[k][:, P * j:P * (j + 1)],
                            attn_sb[k][:, 512 * th:512 * (th + 1)],
                            start=(k == 0), stop=(k == CT - 1))
                    nc.vector.scalar_tensor_tensor(
                        out=ot[:, 512 * th:512 * (th + 1)], in0=ps[:],
                        scalar=bp_sb[:, j:j + 1],
                        in1=x_sb[j][:, 512 * th:512 * (th + 1)],
                        op0=ALU.add, op1=ALU.add)
                    if b == 0:
                        if th == 1:
                            nc.gpsimd.dma_start(
                                out_d[b, P * j:P * (j + 1), :], ot[:])
                    else:
                        # tail batch: store each half as soon as it is
                        # ready, round-robin over all three DMA-capable
                        # queues so the tail drains fast
                        eng = (nc.sync, nc.scalar, nc.gpsimd)[(2 * j + th) % 3]
                        eng.dma_start(
                            out_d[b, P * j:P * (j + 1),
                                  512 * th:512 * (th + 1)],
                            ot[:, 512 * th:512 * (th + 1)])
                return emit

            if th_major:
                order = [(j, th) for th in range(TH) for j in range(CT)]
            else:
                order = [(j, th) for j in range(CT) for th in range(TH)]
            for j, th in order:
                groups.append(pgroup(j, th))
            return groups

        def emit_lg_exp(qk_sb, p_i, st, th, wts):
            """One logits tile [s=128, (2 heads x 512 t)] + one exp.
            The two heads' K=64 matmuls sit at PE row-tiles (0,0)/(64,0)
            and execute concurrently."""
            qt = qk_sb[2 * p_i]
            kt = qk_sb[2 * p_i + 1]
            lg = ps_lg.tile([P, T], F32, tag="pslg")
            for hh in range(2):
                lo = 64 * hh
                nc.tensor.matmul(
                    lg[:, 512 * hh:512 * (hh + 1)],
                    kt[lo:lo + DH, P * st:P * (st + 1)],
                    qt[lo:lo + DH, 512 * th:512 * (th + 1)],
                    start=True, stop=True)
            wt = wtpool.tile([P, T], BF16, tag="wt")
            nc.scalar.activation(wt[:], lg[:], EXP, bias=0.0)
            wts[(st, th)] = wt

        # ---------- software-pipelined schedule ----------
        pair_ids = [(b, p) for b in range(BPC) for p in range(NPAIR)]

        xn0 = emit_gn(x0)
        qk0, vt0 = [], []
        qkv0 = qkv_groups(xn0, qk0, vt0, pair0_first=True)
        for g in qkv0[:4]:      # qk j0, j1 — unblocks pair-0 logits
            g()
        # prefetch logits/exp of the first pair (ahead of v/qk-rest in the
        # in-order tensor queue so ScalarE starts as early as possible)
        wts_cur = {}
        for th in range(TH):
            for st in range(ST):
                emit_lg_exp(qk0, 0, st, th, wts_cur)
        for g in qkv0[4:12]:    # vt st0-7 (needed by the first attn slot)
            g()
        # batch-1 GroupNorm emitted only now so its DVE work lands behind
        # the startup-critical GN0 -> qk -> logits chain in the static order
        xn1 = emit_gn(x1)
        qk1, vt1 = [], []
        # qk j2..j7 of batch 0 + all of batch-1 qkv run as slot fillers
        fillers = qkv0[12:] + qkv_groups(xn1, qk1, vt1, pair0_first=False)

        xs = {0: x0, 1: x1}
        qks = {0: qk0, 1: qk1}
        vts = {0: vt0, 1: vt1}
        attns = {0: [], 1: []}
        pending = []          # deferred divide work of the previous pair

        for idx, (b, p_i) in enumerate(pair_ids):
            last = idx == len(pair_ids) - 1
            vt_sb = vts[b]
            at = attnpool.tile([P, T], BF16, tag="attn")
            recF = recpool.tile([65, T], F32, tag="recF")
            nc.gpsimd.memset(recF[:], 1.0)
            recR = recpool.tile([65, T], F32, tag="recR")
            recB = recpool.tile([65, T], BF16, tag="recB")

            def div_chain(th):
                sl = slice(512 * th, 512 * (th + 1))
                rb = ps_mm.tile([P, 512], F32, tag="psmm")
                nc.tensor.matmul(rb[:], ir_sb[:], recB[:, sl],
                                 start=True, stop=True)
                nc.vector.tensor_mul(at[:, sl], at[:, sl], rb[:])

            wts_next = {}
            nb, np_i = (pair_ids[idx + 1]
                        if idx + 1 < len(pair_ids) else (None, None))
            # next-pair (st, th) emission order: all of th0 during the first
            # two slots (matches wt-tile death of the current pair's th0)
            SLOT_LG = [[(st, 0) for st in range(4)],
                       [(st, 0) for st in range(4, 8)],
                       [(st, 1) for st in range(4)],
                       [(st, 1) for st in range(4, 8)]]

            slot = 0
            for th in range(TH):
                sl = slice(512 * th, 512 * (th + 1))
                for hh in range(2):
                    h_abs = 2 * p_i + hh
                    # make sure the next pair's q/k tiles exist before any
                    # emit_lg_exp references them (batch-boundary case)
                    while (nb is not None and fillers
                           and len(qks[nb]) < 2 * np_i + 2):
                        fillers.pop(0)()
                    lgq = list(SLOT_LG[slot]) if nb is not None else []
                    pa = ps_at.tile([65, 512], F32, tag="psat")
                    for st in range(ST):
                        # interleave next-pair logits+exp in groups of two —
                        # enough to keep ScalarE fed while halving the
                        # PE row-mode (64x128 <-> 128x128) switch count
                        if st in (0, 4):
                            for _ in range(2):
                                if lgq:
                                    st2, th2 = lgq.pop(0)
                                    emit_lg_exp(qks[nb], np_i, st2, th2,
                                                wts_next)
                        nc.tensor.matmul(
                            pa[:],
                            vt_sb[st][:, 65 * h_abs:65 * (h_abs + 1)],
                            wts_cur[(st, th)][:, 512 * hh:512 * (hh + 1)],
                            start=(st == 0), stop=(st == ST - 1))
                    nc.vector.tensor_copy(at[64 * hh:64 * hh + DH, sl],
                                          pa[0:DH, :])
                    nc.vector.tensor_copy(recF[64 * hh:64 * hh + 1, sl],
                                          pa[DH:DH + 1, :])
                    # deferred divides of the previous pair (all at once —
                    # partial pops can deadlock the in-order tensor queue
                    # against filler groups that read the attn tiles)
                    while pending:
                        pending.pop(0)()
                    npop = 2 if len(fillers) > 8 else 1
                    for _ in range(npop):
                        if fillers:
                            fillers.pop(0)()
                    slot += 1
                    # last pair: divide the t0 half as soon as both heads'
                    # t0 slots are done, then start batch-1 proj on that
                    # half under the cover of the t1 attention slots
                    if last and slot == 2:
                        attns[b].append(at)
                        nc.vector.reciprocal_approx_fast(
                            out=recR[:, 0:512], in_=recF[:, 0:512])
                        nc.vector.tensor_copy(recB[:, 0:512], recR[:, 0:512])
                        div_chain(0)
                        pg1 = proj_groups(1, attns[1], x1, th_major=True)
                    if last and slot == 3:
                        for g in pg1[:CT]:
                            g()

            if not last:
                attns[b].append(at)

            if last:
                nc.vector.reciprocal_approx_fast(
                    out=recR[:, 512:T], in_=recF[:, 512:T])
                nc.vector.tensor_copy(recB[:, 512:T], recR[:, 512:T])
                div_chain(1)
                for g in pg1[CT:]:
                    g()
            else:
                # denominator reciprocal now; broadcast matmul + divide
                # deferred into the next pair's slots (keeps the tensor
                # queue stall-free)
                nc.vector.reciprocal_approx_fast(out=recR[:], in_=recF[:])
                nc.vector.tensor_copy(recB[:], recR[:])

                def make_div(at, recB):
                    def run():
                        for th in range(TH):
                            sl = slice(512 * th, 512 * (th + 1))
                            rb = ps_mm.tile([P, 512], F32, tag="psmm")
                            nc.tensor.matmul(rb[:], ir_sb[:], recB[:, sl],
                                             start=True, stop=True)
                            nc.vector.tensor_mul(at[:, sl], at[:, sl], rb[:])
                    return run
                pending.append(make_div(at, recB))
                wts_cur = wts_next

            # end of batch 0's pairs: flush remaining fillers, then queue
            # batch-0 proj as fillers for batch 1's pairs
            if b == 0 and p_i == NPAIR - 1:
                for g in fillers:
                    g()
                fillers = list(proj_groups(0, attns[0], x0))

        # flush any leftover filler work
        for g in fillers:
            g()

    nc.compile()
    return nc


def prep_inputs(x, gn_scale, gn_bias, w_qkv, b_qkv, w_proj, b_proj):
    """Host-side: reorder/scale weights, build per-core input maps."""
    x2 = np.ascontiguousarray(
        np.asarray(x, dtype=np.float32).reshape(B, C, T))
    w_qkv = np.asarray(w_qkv, dtype=np.float32)
    b_qkv = np.asarray(b_qkv, dtype=np.float32)
    scale = float(DH) ** -0.25

    qk_rows = []
    for p_i in range(NPAIR):
        for hh in range(2):           # q rows of the pair
            h = 2 * p_i + hh
            qk_rows.extend(range(192 * h, 192 * h + DH))
        for hh in range(2):           # k rows of the pair
            h = 2 * p_i + hh
            qk_rows.extend(range(192 * h + DH, 192 * h + 2 * DH))
    qk_rows = np.array(qk_rows)
    bf16 = mybir.dt.np(BF16)
    x2 = x2.astype(bf16)
    wqkT = np.ascontiguousarray((w_qkv[qk_rows] * scale).T).astype(bf16)
    bqkT = np.ascontiguousarray(
        (b_qkv[qk_rows] * scale).reshape(OT, P).T)

    v_rows = np.array([192 * h + 2 * DH + j for h in range(NHEADS)
                       for j in range(DH)])
    wvT = np.ascontiguousarray(w_qkv[v_rows].T).astype(bf16)
    bv = b_qkv[v_rows]
    bvA = np.zeros(NHEADS * 65, np.float32)
    for h in range(NHEADS):
        bvA[65 * h:65 * h + DH] = bv[DH * h:DH * (h + 1)]
        bvA[65 * h + DH] = 1.0

    wpT = np.ascontiguousarray(np.asarray(w_proj, np.float32).T).astype(bf16)
    bpT = np.ascontiguousarray(
        np.asarray(b_proj, np.float32).reshape(CT, P).T)
    gnT = np.ascontiguousarray(np.stack(
        [np.asarray(gn_scale, np.float32).reshape(CT, P).T,
         np.asarray(gn_bias, np.float32).reshape(CT, P).T],
        axis=2).reshape(P, CT * 2))

    i8 = np.zeros((P, 8), np.float32)
    for p in range(P):
        i8[p, p // GSIZE] = 1.0
    ib = np.ascontiguousarray(i8.T)
    ir = np.zeros((65, P), np.float32)
    ir[0, 0:64] = 1.0
    ir[64, 64:128] = 1.0
    ir = ir.astype(bf16)

    common = dict(wqkT=wqkT, bqkT=bqkT, wvT=wvT, bvA=bvA, wpT=wpT,
                  bpT=bpT, gnT=gnT, i8=i8, ib=ib, ir=ir)
    in_maps = [dict(common, xs=np.ascontiguousarray(x2[BPC * i:BPC * (i + 1)]))
               for i in range(N_CORES)]
    return in_maps


_NC = None


def _ensure_ntff_hook():
    """The agent image's antenv lacks axon_hooks; shim it and register the
    ctypes NTFF hook from the boot script so trace=True can measure HW time."""
    try:
        from antenv import axon_hooks  # noqa: F401
        return
    except ImportError:
        pass
    import types
    import antenv
    mod = types.ModuleType("antenv.axon_hooks")
    _state = {"fn": None}
    mod.set_axon_ntff_profile_hook = lambda fn: _state.__setitem__("fn", fn)
    mod.get_axon_ntff_profile_hook = lambda: _state["fn"]
    sys.modules["antenv.axon_hooks"] = mod
    antenv.axon_hooks = mod
    try:
        from trn_agent_boot.trn_boot import _ntff_profile_via_ctypes
        hook = _ntff_profile_via_ctypes("/opt/axon/libaxon_pjrt.so")
        mod.set_axon_ntff_profile_hook(hook)
    except Exception as e:  # degrade: run proceeds untraced
        print("ntff hook setup failed:", e)


def kernel(x, gn_scale, gn_bias, w_qkv, b_qkv, w_proj, b_proj):
    global _NC, LAST_RESULTS
    if _NC is None:
        _NC = build_nc()
    in_maps = prep_inputs(x, gn_scale, gn_bias, w_qkv, b_qkv, w_proj, b_proj)
    trace = bool(os.environ.get("KERNEL_TRACE"))
    if trace:
        _ensure_ntff_hook()
    res = run_bass_kernel_spmd(_NC, in_maps, list(range(N_CORES)), trace=trace)
    LAST_RESULTS = res
    out = np.concatenate([res.results[i]["out"] for i in range(N_CORES)],
                         axis=0)
    return out.reshape(B, C, HH, WW).astype(np.float32)
